# revision 43
# baseline (speedup 1.0000x reference)
"""AdaBiRealBasicBlock on 8 TRN2 NeuronCores.

Data-parallel over batch (32 -> 4 images/core), weights replicated.
BN statistics are globally synced with small AllReduces, with all host-
side-precomputable work (weight binarization, hi/lo fp16 split of x,
padding, BN eps folding) moved off-device.

Math:
  b = where(w > tau, +1, -1);  alpha = mean|w| per out-channel
  conv(x, alpha*b) = alpha * conv(x, b)
  BN(alpha*c) then sign == Sign(s*c + t) with
      s = gamma*rsqrt(var_c + eps/alpha^2),  t = beta - s*mean_c

conv1 streams x as two fp16 planes (x_hi = fp16(x), x_lo =
fp16((x-x_hi)*2^12)) against weight planes +-1 and +-2^-12 -- all
exactly representable in fp16, accumulated in fp32 PSUM, so c1 matches
a plain f32 conv to ~1e-7.  conv2 is exact in fp8 (+-1 inputs and
weights) using DoubleRow perf mode: both 128-channel halves contract
in a single matmul pass at 2x column rate.

Engine-queue layout (avoids the baseline's semaphore-backlog stalls):
  tensor : matmuls
  vector : conv epilogues, stats reduces, residual precompute,
           BN finalize chains, final scale+residual (stt)
  scalar : input DMA issue, weight-plane derives, epilogue squares,
           all Sign activations
  gpsimd : y-pad borders, stats DMA -> collective triggers (only!)
  sync   : weight/param DMA issue, AllReduce readbacks, output DMAs

Collectives: 3 total. AR1 (layer-1 co0 stats) fully hidden under
conv1-co1; AR2 (layer-1 co1) on the critical path but triggered
immediately; AR3 merges both layer-2 stat groups (2KB).
"""
import os
import sys

if "/opt/trn_rl_repo" not in sys.path:
    sys.path.insert(0, "/opt/trn_rl_repo")

import numpy as np

import concourse.bass as bass
import concourse.bacc as bacc
import concourse.mybir as mybir
from concourse.tile import TileContext
from concourse import bass_utils

F32 = mybir.dt.float32
FP16 = mybir.dt.float16
FP8 = mybir.dt.float8e4
AF = mybir.ActivationFunctionType
ALU = mybir.AluOpType
AX = mybir.AxisListType
DR_MODE = mybir.MatmulPerfMode.DoubleRow

B, C, H, W = 32, 256, 28, 28
NCORES = 8
BL = B // NCORES            # images per core
HP, WP = H + 2, W + 2       # padded 30x30
IMG = HP * WP               # 900
SP = BL * H * W             # 3136 spatial elements per core
KTAPS = 9
KW = C * KTAPS              # 2304 contraction
EPS = 1e-5
NTOT = float(B * H * W)     # global BN count
CHUNKS = [(i, h0) for i in range(BL) for h0 in (0, H // 2)]  # 8 x [14 rows]
CH_R = H // 2               # 14 rows per chunk
CH_N = CH_R * W             # 392
HB = BL // 2                # half-batch images
HSP = SP // 2

_NC_CACHE = {}
LAST_RESULT = None
USE_FP8 = True
LDW_OPT = os.environ.get("KLDW", "0") == "1"
NEWTON = os.environ.get("KNEWT", "0") == "1"


def _patch_ldw_opt():
    """walrus is invoked with --enable-ldw-opt=false by default; flipping it
    lets codegen elide/overlap redundant LDWEIGHTS (consecutive matmuls
    share weights in the tap-outer conv phases)."""
    if getattr(bass_utils, "_ldw_patched", False):
        return
    orig = bass_utils.run_command

    def patched(cmd, *a, **kw):
        if LDW_OPT and isinstance(cmd, list):
            cmd = ["--enable-ldw-opt=true" if c == "--enable-ldw-opt=false"
                   else c for c in cmd]
        return orig(cmd, *a, **kw)

    bass_utils.run_command = patched
    bass_utils._ldw_patched = True


def _build_nc():
    nc = bacc.Bacc("TRN2", target_bir_lowering=False, debug=False,
                   num_devices=NCORES)

    xhi_d = nc.declare_dram_parameter("xhi", [128, 2, BL, HP, WP], FP16,
                                      isOutput=False)
    xlo_d = nc.declare_dram_parameter("xlo", [128, 2, BL, HP, WP], FP16,
                                      isOutput=False)
    w1b_d = nc.declare_dram_parameter("w1b", [128, 2, KTAPS, C], FP16,
                                      isOutput=False)
    w2b_d = nc.declare_dram_parameter("w2b", [128, 2, KTAPS, C], FP16,
                                      isOutput=False)
    par_d = nc.declare_dram_parameter("par", [128, 12], F32, isOutput=False)
    out_d = nc.declare_dram_parameter("out", [BL, C, H, W], FP16,
                                      isOutput=True)

    with TileContext(nc) as tc:
        with (
            tc.tile_pool(name="main", bufs=1) as P,
            tc.tile_pool(name="sqpool", bufs=2) as SQ,
            tc.tile_pool(name="psum", bufs=1, space="PSUM") as PS,
            tc.tile_pool(name="dram", bufs=1, space="DRAM") as DRm,
        ):
            # ---- persistent tiles ----
            x_hi = P.tile([128, 2 * BL * IMG], FP16, name="x_hi")
            x_lo = P.tile([128, 2 * BL * IMG], FP16, name="x_lo")
            w1b = P.tile([128, 2 * KW], FP16, name="w1b")
            w1bl = P.tile([128, 2 * KW], FP16, name="w1bl")
            w2b16 = P.tile([128, 2 * KW], FP16, name="w2b16")
            y8 = P.tile([128, 2 * BL * IMG], FP8 if USE_FP8 else FP16,
                        name="y8")
            w2b8 = P.tile([128, 2 * KW], FP8 if USE_FP8 else FP16,
                          name="w2b8")
            c1 = [P.tile([128, SP], F32, name=f"c1_{k}") for k in range(2)]
            c2 = [P.tile([128, SP], F32, name=f"c2_{k}") for k in range(2)]
            res = [P.tile([128, SP], F32, name=f"res{k}") for k in range(2)]
            o16 = [P.tile([128, SP], FP16, name=f"o16_{k}") for k in range(2)]
            par = P.tile([128, 12], F32, name="par")
            sums1 = P.tile([128, 16], F32, name="sums1")
            ssq1 = P.tile([128, 16], F32, name="ssq1")
            sums2 = P.tile([128, 16], F32, name="sums2")
            ssq2 = P.tile([128, 16], F32, name="ssq2")
            st1 = [P.tile([128, 2], F32, name=f"st1_{a}") for a in range(2)]
            st2 = [P.tile([128, 2], F32, name=f"st2_{a}") for a in range(2)]
            fin1 = P.tile([128, 4], F32, name="fin1")
            fin2 = P.tile([128, 4], F32, name="fin2")
            gate1 = P.tile([128, 1], F32, name="gate1")
            s1c = P.tile([128, 2], F32, name="s1c")
            t1c = P.tile([128, 2], F32, name="t1c")
            s2c = P.tile([128, 2], F32, name="s2c")
            t2c = P.tile([128, 2], F32, name="t2c")
            fsc = P.tile([128, 24], F32, name="fsc")  # finalize scratch

            cc_in = [DRm.tile([128, 2], F32, name=f"cc_in{j}")
                     for j in range(2)]
            cc_out = [DRm.tile([128, 2], F32, addr_space="Shared",
                               name=f"cc_out{j}") for j in range(2)]
            cc_in2 = DRm.tile([128, 4], F32, name="cc_in2")
            cc_out2 = DRm.tile([128, 4], F32, addr_space="Shared",
                               name="cc_out2")

            xhiv = x_hi.rearrange("p (k i h w) -> p k i h w",
                                  k=2, i=BL, h=HP, w=WP)
            xlov = x_lo.rearrange("p (k i h w) -> p k i h w",
                                  k=2, i=BL, h=HP, w=WP)
            y8v = y8.rearrange("p (k i h w) -> p k i h w",
                               k=2, i=BL, h=HP, w=WP)
            w1v = w1b.rearrange("p (k t o) -> p k t o", k=2, t=KTAPS)
            w1lv = w1bl.rearrange("p (k t o) -> p k t o", k=2, t=KTAPS)
            w2v = w2b8.rearrange("p (k t o) -> p k t o", k=2, t=KTAPS)

            # ---- prologue DMAs, ordered by first use; the sync queue arms
            # earliest so the conv1-co0 critical cone goes there ----
            xsrc_hi = xhi_d.ap()
            xsrc_lo = xlo_d.ap()
            w1bsb = w1b.rearrange("p (k f) -> p k f", k=2)
            w1bdr = w1b_d.ap().rearrange("p k t o -> p k (t o)")
            # first matmul needs w1b[k0, tap0] and image 0 only: lead with
            # the smallest sufficient pieces
            # first-matmul cone split across BOTH hardware-DGE queues so the
            # weight and activation transfers land in parallel
            T3 = 3 * C
            nc.sync.dma_start(out=w1bsb[:, 0, 0:T3], in_=w1bdr[:, 0, 0:T3])
            nc.scalar.dma_start(out=xhiv[:, 0, 0:HB], in_=xsrc_hi[:, 0, 0:HB])
            nc.sync.dma_start(out=w1bsb[:, 0, T3:KW], in_=w1bdr[:, 0, T3:KW])
            nc.scalar.dma_start(out=xhiv[:, 0, HB:BL],
                                in_=xsrc_hi[:, 0, HB:BL])
            nc.sync.dma_start(out=w1bsb[:, 1], in_=w1bdr[:, 1])
            for hh in range(2):
                isl = slice(hh * HB, (hh + 1) * HB)
                nc.sync.dma_start(out=xhiv[:, 1, isl], in_=xsrc_hi[:, 1, isl])
            nc.scalar.dma_start(out=w2b16[:, :],
                                in_=w2b_d.ap().rearrange(
                                    "p k t o -> p (k t o)"))
            nc.scalar.dma_start(out=par[:, :], in_=par_d.ap())
            for k in range(2):
                for hh in range(2):
                    isl = slice(hh * HB, (hh + 1) * HB)
                    nc.scalar.dma_start(out=xlov[:, k, isl],
                                        in_=xsrc_lo[:, k, isl])

            # derived weight planes (scalar; off critical path)
            nc.scalar.activation(out=w1bl[:, :], in_=w1b[:, :],
                                 func=AF.Copy, scale=2.0 ** -12)
            nc.scalar.activation(out=w2b8[:, :], in_=w2b16[:, :],
                                 func=AF.Copy)

            # y-pad borders (gpsimd; idle otherwise)
            nc.gpsimd.memset(y8v[:, :, :, 0, :], 0.0)
            nc.gpsimd.memset(y8v[:, :, :, HP - 1, :], 0.0)
            nc.gpsimd.memset(y8v[:, :, :, 1:HP - 1, 0], 0.0)
            nc.gpsimd.memset(y8v[:, :, :, 1:HP - 1, WP - 1], 0.0)

            gcols = {1: (par[:, 0:2], par[:, 2:4], par[:, 4:6]),
                     2: (par[:, 6:8], par[:, 8:10], par[:, 10:12])}

            # ---- conv builders ----
            def conv_co(tag, planes, co, csb, sums, ssq):
                """planes: list of (weight_view[p,t*?o slice fn], moving
                view[p,i,h,w]) matmul'd in fp16."""
                NP = len(planes)
                pss = [PS.tile([128, CH_N], F32, tag=f"ps{ch}",
                               name=f"ps_{tag}_{co}_{ch}")
                       for ch in range(8)]

                def epilogue(ch):
                    cs = csb[co][:, ch * CH_N:(ch + 1) * CH_N]
                    sl = co * 8 + ch
                    nc.vector.tensor_scalar(
                        out=cs, in0=pss[ch][:, :], scalar1=0.0,
                        scalar2=0.0, op0=ALU.add, op1=ALU.add,
                        accum_out=sums[:, sl:sl + 1])
                    sq = SQ.tile([128, CH_N], F32, tag="sq",
                                 name=f"sq_{tag}_{co}_{ch}")
                    nc.scalar.activation(
                        out=sq[:, :], in_=cs, func=AF.Square,
                        accum_out=ssq[:, sl:sl + 1])

                def emit(k, t, ch, im, h0):
                    dy, dx = t // 3, t % 3
                    wview, mview = planes[k]
                    wap = wview[:, t, co * 128:co * 128 + 128]
                    first = (k == 0 and t == 0)
                    last = (k == NP - 1 and t == KTAPS - 1)
                    mov = mview[:, im, h0 + dy:h0 + dy + CH_R, dx:dx + W]
                    nc.tensor.matmul(pss[ch][:, :], wap, mov,
                                     start=first, stop=last)

                for k in range(NP - 1):
                    for t in range(KTAPS):
                        for ch, (im, h0) in enumerate(CHUNKS):
                            emit(k, t, ch, im, h0)
                for ch, (im, h0) in enumerate(CHUNKS):
                    for t in range(KTAPS):
                        emit(NP - 1, t, ch, im, h0)
                    epilogue(ch)

            def c2_epilogue(co, pss, ch):
                cs = c2[co][:, ch * CH_N:(ch + 1) * CH_N]
                sl = co * 8 + ch
                nc.vector.tensor_scalar(
                    out=cs, in0=pss[ch][:, :], scalar1=0.0,
                    scalar2=0.0, op0=ALU.add, op1=ALU.add,
                    accum_out=sums2[:, sl:sl + 1])
                sq = SQ.tile([128, CH_N], F32, tag="sq",
                             name=f"sq_c2_{co}_{ch}")
                nc.scalar.activation(
                    out=sq[:, :], in_=cs, func=AF.Square,
                    accum_out=ssq2[:, sl:sl + 1])

            def conv2_co_fp8(co):
                """DoubleRow fp8: both 128ch halves contracted per pass."""
                pss = [PS.tile([128, CH_N], F32, tag=f"ps{ch}",
                               name=f"ps_c2_{co}_{ch}")
                       for ch in range(8)]
                for ch, (im, h0) in enumerate(CHUNKS):
                    for t in range(KTAPS):
                        dy, dx = t // 3, t % 3
                        wap = w2v[:, :, t, co * 128:co * 128 + 128]
                        mov = y8v[:, :, im, h0 + dy:h0 + dy + CH_R,
                                  dx:dx + W]
                        nc.tensor.matmul(pss[ch][:, :], wap, mov,
                                         start=(t == 0), stop=(t == KTAPS - 1),
                                         perf_mode=DR_MODE)
                    c2_epilogue(co, pss, ch)

            def conv2_co_split(co):
                """Single-k fp8 matmuls: the k=0 half only needs y[co=0], so
                it runs during the AR2/y-co1 wait; k=1 continues the groups."""
                pss = [PS.tile([128, CH_N], F32, tag=f"ps{ch}",
                               name=f"ps_c2_{co}_{ch}")
                       for ch in range(8)]
                for t in range(KTAPS):
                    dy, dx = t // 3, t % 3
                    wap = w2v[:, 0, t, co * 128:co * 128 + 128]
                    for ch, (im, h0) in enumerate(CHUNKS):
                        mov = y8v[:, 0, im, h0 + dy:h0 + dy + CH_R, dx:dx + W]
                        nc.tensor.matmul(pss[ch][:, :], wap, mov,
                                         start=(t == 0), stop=False)
                for ch, (im, h0) in enumerate(CHUNKS):
                    for t in range(KTAPS):
                        dy, dx = t // 3, t % 3
                        wap = w2v[:, 1, t, co * 128:co * 128 + 128]
                        mov = y8v[:, 1, im, h0 + dy:h0 + dy + CH_R, dx:dx + W]
                        nc.tensor.matmul(pss[ch][:, :], wap, mov,
                                         start=False, stop=(t == KTAPS - 1))
                    c2_epilogue(co, pss, ch)

            # ---- stats: reduce (vector) -> DMA (gpsimd) -> AR (gpsimd) ----
            def stats1_co(co):
                nc.vector.reduce_sum(out=st1[co][:, 0:1],
                                     in_=sums1[:, co * 8:(co + 1) * 8],
                                     axis=AX.X)
                nc.vector.reduce_sum(out=st1[co][:, 1:2],
                                     in_=ssq1[:, co * 8:(co + 1) * 8],
                                     axis=AX.X)
                # stats DMA on scalar = hardware DGE (~1us, vs gpsimd's
                # ~4us soft DGE) and NOT on sync: the sync queue holds the
                # AR readbacks, which complete at skew-inflated times — the
                # next AR's trigger must never chain behind them
                nc.scalar.dma_start(out=cc_in[co][:, :], in_=st1[co][:, :])
                nc.gpsimd.collective_compute(
                    "AllReduce", ALU.add,
                    replica_groups=[list(range(NCORES))],
                    ins=[cc_in[co].opt()], outs=[cc_out[co].opt()])
                # readback on sync (NOT gpsimd: keeps the next trigger free)
                nc.sync.dma_start(out=fin1[:, 2 * co:2 * co + 2],
                                  in_=cc_out[co][:, :])

            # ---- BN finalize: s,t columns (vector + one scalar sqrt) ----
            # co=None batches both co groups in 2-wide column ops
            def finalize_co(lay, co, fin, s_out, t_out, base, gate=None):
                gcol, bcol, ecol = gcols[lay]
                if co is None:
                    fv = fin.rearrange("p (c two) -> p c two", two=2)
                    Ssum = fv[:, :, 0]
                    Ssq = fv[:, :, 1]
                    nw = 2
                    so = s_out[:, :]
                    to = t_out[:, :]
                else:
                    Ssum = fin[:, 2 * co:2 * co + 1]
                    Ssq = fin[:, 2 * co + 1:2 * co + 2]
                    nw = 1
                    so = s_out[:, co:co + 1]
                    to = t_out[:, co:co + 1]
                    gcol = gcol[:, co:co + 1]
                    bcol = bcol[:, co:co + 1]
                    ecol = ecol[:, co:co + 1]
                mean = fsc[:, base + 0:base + nw]
                msq = fsc[:, base + 4:base + 4 + nw]
                u = fsc[:, base + 8:base + 8 + nw]
                tmp = fsc[:, base + 12:base + 12 + nw]
                if gate is not None:
                    # gate==0; its only purpose is a data dependency that
                    # keeps the Tile scheduler (whose sim has no idea how
                    # slow the AllReduce really is) from sequencing these
                    # chain heads ahead of still-pending conv epilogues.
                    # BOTH readback-reading head ops must be gated.
                    nc.vector.scalar_tensor_tensor(
                        out=mean, in0=Ssum, scalar=1.0 / NTOT,
                        in1=gate, op0=ALU.mult, op1=ALU.add)
                    nc.vector.scalar_tensor_tensor(
                        out=msq, in0=Ssq, scalar=1.0 / NTOT,
                        in1=gate, op0=ALU.mult, op1=ALU.add)
                else:
                    nc.vector.tensor_scalar(out=mean, in0=Ssum,
                                            scalar1=1.0 / NTOT,
                                            scalar2=None, op0=ALU.mult)
                    nc.vector.tensor_scalar(out=msq, in0=Ssq,
                                            scalar1=1.0 / NTOT,
                                            scalar2=None, op0=ALU.mult)
                nc.vector.tensor_tensor(out=u, in0=mean, in1=mean, op=ALU.mult)
                nc.vector.tensor_tensor(out=u, in0=msq, in1=u,
                                        op=ALU.subtract)
                nc.vector.tensor_tensor(out=u, in0=u, in1=ecol, op=ALU.add)
                # rsqrt = sqrt(1/u) + one Newton step
                nc.vector.reciprocal(out=so, in_=u)
                nc.scalar.activation(out=so, in_=so, func=AF.Sqrt)
                if NEWTON:
                    nc.vector.tensor_tensor(out=tmp, in0=so, in1=so,
                                            op=ALU.mult)
                    nc.vector.tensor_tensor(out=tmp, in0=tmp, in1=u,
                                            op=ALU.mult)
                    nc.vector.tensor_scalar(out=tmp, in0=tmp, scalar1=-0.5,
                                            scalar2=1.5, op0=ALU.mult,
                                            op1=ALU.add)
                    nc.vector.tensor_tensor(out=so, in0=so, in1=tmp,
                                            op=ALU.mult)
                nc.vector.tensor_tensor(out=so, in0=so, in1=gcol,
                                        op=ALU.mult)
                nc.vector.tensor_tensor(out=tmp, in0=so, in1=mean,
                                        op=ALU.mult)
                nc.vector.tensor_tensor(out=to, in0=bcol, in1=tmp,
                                        op=ALU.subtract)

            def y1_sign(co):
                # conv2's first chunk (img0 rows 0-13, all taps) needs y
                # rows 0-14 only: lead with a 16-row piece so the dependent
                # matmuls un-gate ~0.5us earlier, then img0's tail, then
                # the remaining images whole
                src = c1[co].rearrange("p (i h w) -> p i h w", i=BL, h=H, w=W)
                pieces = [(0, 0, 16), (0, 16, H)] + [(im, 0, H)
                                                    for im in range(1, BL)]
                for im, r0, r1 in pieces:
                    dst = y8v[:, co, im, 1 + r0:1 + r1, 1:WP - 1]
                    nc.scalar.activation(out=dst, in_=src[:, im, r0:r1],
                                         func=AF.Sign,
                                         bias=t1c[:, co:co + 1],
                                         scale=s1c[:, co:co + 1])

            planes1 = [(w1v[:, 0], xhiv[:, 0]), (w1v[:, 1], xhiv[:, 1]),
                       (w1lv[:, 0], xlov[:, 0]), (w1lv[:, 1], xlov[:, 1])]

            # ================= layer 1 =================
            def res_precompute(co):
                # res = x_hi + 2^-12 * x_lo  (== x to ~2^-22); per image so
                # the strided interior views stay 3D (walrus limit)
                rv = res[co].rearrange("p (i h w) -> p i h w", i=BL, h=H, w=W)
                for im in range(BL):
                    nc.vector.scalar_tensor_tensor(
                        out=rv[:, im],
                        in0=xlov[:, co, im, 1:HP - 1, 1:WP - 1],
                        scalar=2.0 ** -12,
                        in1=xhiv[:, co, im, 1:HP - 1, 1:WP - 1],
                        op0=ALU.mult, op1=ALU.add)

            conv_co("c1", planes1, 0, c1, sums1, ssq1)
            stats1_co(0)
            # residual precompute co0 (fills vector idle during AR1)
            res_precompute(0)
            conv_co("c1", planes1, 1, c1, sums1, ssq1)
            stats1_co(1)
            res_precompute(1)
            # scheduler gate: ready only once conv1-co1's stats are done
            nc.vector.tensor_scalar(out=gate1[:, :], in0=st1[1][:, 0:1],
                                    scalar1=0.0, scalar2=None, op0=ALU.mult)
            finalize_co(1, 0, fin1, s1c, t1c, 0, gate=gate1[:, 0:1])
            y1_sign(0)
            finalize_co(1, 1, fin1, s1c, t1c, 1)
            y1_sign(1)

            # ================= layer 2 =================
            # one merged AllReduce for both co groups: consecutive CC ops
            # pay a ~20us stream re-arm, so a second L2 AR would start
            # re-arm-blocked and cost more than it hides
            def stats2_co(co):
                nc.vector.reduce_sum(out=st2[co][:, 0:1],
                                     in_=sums2[:, co * 8:(co + 1) * 8],
                                     axis=AX.X)
                nc.vector.reduce_sum(out=st2[co][:, 1:2],
                                     in_=ssq2[:, co * 8:(co + 1) * 8],
                                     axis=AX.X)

            outdst = out_d.ap().rearrange("i (k p) h w -> k p i (h w)", k=2)

            def out_epilogue(co, nsplit=2):
                # nsplit=4 for the last co group: tighter stt->Sign->DMA
                # pipeline on the exposed tail
                QSP = SP // nsplit
                QB = BL // nsplit if nsplit <= BL else 1
                for hh in range(nsplit):
                    sl = slice(hh * QSP, (hh + 1) * QSP)
                    nc.vector.scalar_tensor_tensor(
                        out=c2[co][:, sl], in0=c2[co][:, sl],
                        scalar=s2c[:, co:co + 1], in1=res[co][:, sl],
                        op0=ALU.mult, op1=ALU.add)
                    nc.scalar.activation(out=o16[co][:, sl],
                                         in_=c2[co][:, sl],
                                         func=AF.Sign, bias=t2c[:, co:co + 1])
                    nc.sync.dma_start(
                        out=outdst[co][:, hh * QB:(hh + 1) * QB],
                        in_=o16[co][:, sl].rearrange("p (i hw) -> p i hw",
                                                     i=QB))

            conv2_co_split(0)
            stats2_co(0)
            # co0's stat half ships while conv2-co1 still runs
            nc.scalar.dma_start(out=cc_in2[:, 0:2], in_=st2[0][:, :])
            conv2_co_fp8(1)
            stats2_co(1)
            nc.scalar.dma_start(out=cc_in2[:, 2:4], in_=st2[1][:, :])
            nc.gpsimd.collective_compute(
                "AllReduce", ALU.add,
                replica_groups=[list(range(NCORES))],
                ins=[cc_in2.opt()], outs=[cc_out2.opt()])
            nc.sync.dma_start(out=fin2[:, :], in_=cc_out2[:, :])
            finalize_co(2, None, fin2, s2c, t2c, 2)
            out_epilogue(0)
            out_epilogue(1, nsplit=4)

    nc.compile()
    return nc


def _get_nc():
    if "nc" not in _NC_CACHE:
        _patch_ldw_opt()
        _NC_CACHE["nc"] = _build_nc()
    return _NC_CACHE["nc"]


def _host_prep(x, w1, tau1, gamma1, beta1, w2, tau2, gamma2, beta2):
    f = np.float32
    x = np.asarray(x, f)
    w1 = np.asarray(w1, f)
    w2 = np.asarray(w2, f)

    # hi/lo fp16 split, padded, partition-major [cores][128,2,BL,HP,WP]
    xhi = x.astype(np.float16)
    xlo = ((x - xhi.astype(f)) * 4096.0).astype(np.float16)

    def pack_x(a):
        pad = np.zeros((B, C, HP, WP), np.float16)
        pad[:, :, 1:HP - 1, 1:WP - 1] = a
        # [B,C,HP,WP] -> [cores, BL, k, p, HP, WP] -> [cores, p, k, BL, ...]
        r = pad.reshape(NCORES, BL, 2, 128, HP, WP).transpose(0, 3, 2, 1, 4, 5)
        return np.ascontiguousarray(r)

    xhi_p = pack_x(xhi)
    xlo_p = pack_x(xlo)

    def pack_w(w, tau):
        b = np.where(w > np.asarray(tau, f).reshape(C, 1, 1, 1), 1.0, -1.0)
        # [O,I,3,3] -> [I=k*128+p, t, O] -> [p, k, t, O]
        t = b.astype(np.float16).transpose(1, 2, 3, 0).reshape(2, 128,
                                                               KTAPS, C)
        return np.ascontiguousarray(t.transpose(1, 0, 2, 3))

    w1p = pack_w(w1, tau1)
    w2p = pack_w(w2, tau2)

    # fold alpha into eps:  s = gamma * rsqrt(var + eps/alpha^2)
    def epsp(w):
        a = np.abs(w.astype(np.float64)).mean(axis=(1, 2, 3))
        return (EPS / (a * a)).astype(f)

    cols = [np.asarray(gamma1, f).reshape(C), np.asarray(beta1, f).reshape(C),
            epsp(w1),
            np.asarray(gamma2, f).reshape(C), np.asarray(beta2, f).reshape(C),
            epsp(w2)]
    par = np.zeros((128, 12), f)
    for j, col in enumerate(cols):
        par[:, 2 * j] = col[0:128]
        par[:, 2 * j + 1] = col[128:256]

    return xhi_p, xlo_p, w1p, w2p, par


def kernel(x, w1, tau1, gamma1, beta1, w2, tau2, gamma2, beta2,
           trace=False, trace_kwargs=None):
    global LAST_RESULT
    xhi_p, xlo_p, w1p, w2p, par = _host_prep(
        x, w1, tau1, gamma1, beta1, w2, tau2, gamma2, beta2)
    common = {"w1b": w1p, "w2b": w2p, "par": par}
    in_maps = [
        {"xhi": xhi_p[i], "xlo": xlo_p[i], **common}
        for i in range(NCORES)
    ]
    nc = _get_nc()
    kwargs = {}
    if trace:
        kwargs["trace"] = True
        if trace_kwargs:
            kwargs.update(trace_kwargs)
    res = bass_utils.run_bass_kernel_spmd(nc, in_maps,
                                          core_ids=list(range(NCORES)),
                                          **kwargs)
    LAST_RESULT = res
    out = np.concatenate([res.results[i]["out"] for i in range(NCORES)],
                         axis=0)
    return out.astype(np.float32)


# revision 52
# speedup vs baseline: 1.2968x; 1.2968x over previous
"""AdaBiRealBasicBlock on 8 TRN2 NeuronCores.

Data-parallel over batch (32 -> 4 images/core), weights replicated.
BN statistics are globally synced with small AllReduces, with all host-
side-precomputable work (weight binarization, hi/lo fp16 split of x,
padding, BN eps folding) moved off-device.

Math:
  b = where(w > tau, +1, -1);  alpha = mean|w| per out-channel
  conv(x, alpha*b) = alpha * conv(x, b)
  BN(alpha*c) then sign == Sign(s*c + t) with
      s = gamma*rsqrt(var_c + eps/alpha^2),  t = beta - s*mean_c

conv1 streams x as two fp16 planes (x_hi = fp16(x), x_lo =
fp16((x-x_hi)*2^12)) against weight planes +-1 and +-2^-12 -- all
exactly representable in fp16, accumulated in fp32 PSUM, so c1 matches
a plain f32 conv to ~1e-7.  conv2 is exact in fp8 (+-1 inputs and
weights) using DoubleRow perf mode: both 128-channel halves contract
in a single matmul pass at 2x column rate.

Engine-queue layout (avoids the baseline's semaphore-backlog stalls):
  tensor : matmuls
  vector : conv epilogues, stats reduces, residual precompute,
           BN finalize chains, final scale+residual (stt)
  scalar : input DMA issue, weight-plane derives, epilogue squares,
           all Sign activations
  gpsimd : y-pad borders, stats DMA -> collective triggers (only!)
  sync   : weight/param DMA issue, AllReduce readbacks, output DMAs

Collectives: 3 total. AR1 (layer-1 co0 stats) fully hidden under
conv1-co1; AR2 (layer-1 co1) on the critical path but triggered
immediately; AR3 merges both layer-2 stat groups (2KB).
"""
import os
import sys

if "/opt/trn_rl_repo" not in sys.path:
    sys.path.insert(0, "/opt/trn_rl_repo")

import numpy as np

import concourse.bass as bass
import concourse.bacc as bacc
import concourse.mybir as mybir
from concourse.tile import TileContext
from concourse import bass_utils

F32 = mybir.dt.float32
FP16 = mybir.dt.float16
FP8 = mybir.dt.float8e4
AF = mybir.ActivationFunctionType
ALU = mybir.AluOpType
AX = mybir.AxisListType
DR_MODE = mybir.MatmulPerfMode.DoubleRow

B, C, H, W = 32, 256, 28, 28
NCORES = 8
BL = B // NCORES            # images per core
HP, WP = H + 2, W + 2       # padded 30x30
IMG = HP * WP               # 900
SP = BL * H * W             # 3136 spatial elements per core
KTAPS = 9
KW = C * KTAPS              # 2304 contraction
EPS = 1e-5
NTOT = float(B * H * W)     # global BN count
CHUNKS = [(i, h0) for i in range(BL) for h0 in (0, H // 2)]  # 8 x [14 rows]
CH_R = H // 2               # 14 rows per chunk
CH_N = CH_R * W             # 392
HB = BL // 2                # half-batch images
HSP = SP // 2

_NC_CACHE = {}
LAST_RESULT = None
USE_FP8 = True
LDW_OPT = os.environ.get("KLDW", "0") == "1"
NEWTON = os.environ.get("KNEWT", "0") == "1"


def _patch_ldw_opt():
    """walrus is invoked with --enable-ldw-opt=false by default; flipping it
    lets codegen elide/overlap redundant LDWEIGHTS (consecutive matmuls
    share weights in the tap-outer conv phases)."""
    if getattr(bass_utils, "_ldw_patched", False):
        return
    orig = bass_utils.run_command

    def patched(cmd, *a, **kw):
        if LDW_OPT and isinstance(cmd, list):
            cmd = ["--enable-ldw-opt=true" if c == "--enable-ldw-opt=false"
                   else c for c in cmd]
        return orig(cmd, *a, **kw)

    bass_utils.run_command = patched
    bass_utils._ldw_patched = True


def _build_nc():
    nc = bacc.Bacc("TRN2", target_bir_lowering=False, debug=False,
                   num_devices=NCORES)

    xhi_d = nc.declare_dram_parameter("xhi", [128, 2, BL, HP, WP], FP16,
                                      isOutput=False)
    xlo_d = nc.declare_dram_parameter("xlo", [128, 2, BL, HP, WP], FP16,
                                      isOutput=False)
    w1b_d = nc.declare_dram_parameter("w1b", [128, 2, KTAPS, C], FP16,
                                      isOutput=False)
    w2b_d = nc.declare_dram_parameter("w2b", [128, 2, KTAPS, C], FP16,
                                      isOutput=False)
    par_d = nc.declare_dram_parameter("par", [128, 12], F32, isOutput=False)
    out_d = nc.declare_dram_parameter("out", [BL, C, H, W], FP16,
                                      isOutput=True)

    with TileContext(nc) as tc:
        with (
            tc.tile_pool(name="main", bufs=1) as P,
            tc.tile_pool(name="sqpool", bufs=2) as SQ,
            tc.tile_pool(name="psum", bufs=1, space="PSUM") as PS,
            tc.tile_pool(name="dram", bufs=1, space="DRAM") as DRm,
        ):
            # ---- persistent tiles ----
            x_hi = P.tile([128, 2 * BL * IMG], FP16, name="x_hi")
            x_lo = P.tile([128, 2 * BL * IMG], FP16, name="x_lo")
            w1b = P.tile([128, 2 * KW], FP16, name="w1b")
            w1bl = P.tile([128, 2 * KW], FP16, name="w1bl")
            w2b16 = P.tile([128, 2 * KW], FP16, name="w2b16")
            y8 = P.tile([128, 2 * BL * IMG], FP8 if USE_FP8 else FP16,
                        name="y8")
            w2b8 = P.tile([128, 2 * KW], FP8 if USE_FP8 else FP16,
                          name="w2b8")
            # zero-padded halves: [w2_k0 | 0] and [0 | w2_k1] let conv2-co0
            # run BOTH its phases as full DoubleRow (203ns/mm) instead of
            # single-k (231ns/mm)
            w2z = [P.tile([128, 2 * KW], FP8, name=f"w2z{k}")
                   for k in range(2)]
            c1 = [P.tile([128, SP], F32, name=f"c1_{k}") for k in range(2)]
            c2 = [P.tile([128, SP], F32, name=f"c2_{k}") for k in range(2)]
            res = [P.tile([128, SP], F32, name=f"res{k}") for k in range(2)]
            o16 = [P.tile([128, SP], FP16, name=f"o16_{k}") for k in range(2)]
            par = P.tile([128, 12], F32, name="par")
            sums1 = P.tile([128, 16], F32, name="sums1")
            ssq1 = P.tile([128, 16], F32, name="ssq1")
            sums2 = P.tile([128, 16], F32, name="sums2")
            ssq2 = P.tile([128, 16], F32, name="ssq2")
            st1 = [P.tile([128, 2], F32, name=f"st1_{a}") for a in range(2)]
            st2 = [P.tile([128, 2], F32, name=f"st2_{a}") for a in range(2)]
            fin1 = P.tile([128, 4], F32, name="fin1")
            fin2 = P.tile([128, 4], F32, name="fin2")
            gate1 = P.tile([128, 1], F32, name="gate1")
            s1c = P.tile([128, 2], F32, name="s1c")
            t1c = P.tile([128, 2], F32, name="t1c")
            s2c = P.tile([128, 2], F32, name="s2c")
            t2c = P.tile([128, 2], F32, name="t2c")
            fsc = P.tile([128, 24], F32, name="fsc")  # finalize scratch

            cc_in = [DRm.tile([128, 2], F32, name=f"cc_in{j}")
                     for j in range(2)]
            cc_out = [DRm.tile([128, 2], F32, addr_space="Shared",
                               name=f"cc_out{j}") for j in range(2)]
            cc_in2 = DRm.tile([128, 4], F32, name="cc_in2")
            cc_out2 = DRm.tile([128, 4], F32, addr_space="Shared",
                               name="cc_out2")

            xhiv = x_hi.rearrange("p (k i h w) -> p k i h w",
                                  k=2, i=BL, h=HP, w=WP)
            xlov = x_lo.rearrange("p (k i h w) -> p k i h w",
                                  k=2, i=BL, h=HP, w=WP)
            y8v = y8.rearrange("p (k i h w) -> p k i h w",
                               k=2, i=BL, h=HP, w=WP)
            w1v = w1b.rearrange("p (k t o) -> p k t o", k=2, t=KTAPS)
            w1lv = w1bl.rearrange("p (k t o) -> p k t o", k=2, t=KTAPS)
            w2v = w2b8.rearrange("p (k t o) -> p k t o", k=2, t=KTAPS)

            # ---- prologue DMAs, ordered by first use; the sync queue arms
            # earliest so the conv1-co0 critical cone goes there ----
            xsrc_hi = xhi_d.ap()
            xsrc_lo = xlo_d.ap()
            w1bsb = w1b.rearrange("p (k f) -> p k f", k=2)
            w1bdr = w1b_d.ap().rearrange("p k t o -> p k (t o)")
            # first matmul needs w1b[k0, tap0] and image 0 only: lead with
            # the smallest sufficient pieces
            # first-matmul cone split across BOTH hardware-DGE queues so the
            # weight and activation transfers land in parallel
            T3 = 3 * C
            nc.sync.dma_start(out=w1bsb[:, 0, 0:T3], in_=w1bdr[:, 0, 0:T3])
            nc.scalar.dma_start(out=xhiv[:, 0, 0:HB], in_=xsrc_hi[:, 0, 0:HB])
            nc.sync.dma_start(out=w1bsb[:, 0, T3:KW], in_=w1bdr[:, 0, T3:KW])
            nc.scalar.dma_start(out=xhiv[:, 0, HB:BL],
                                in_=xsrc_hi[:, 0, HB:BL])
            nc.sync.dma_start(out=w1bsb[:, 1], in_=w1bdr[:, 1])
            for hh in range(2):
                isl = slice(hh * HB, (hh + 1) * HB)
                nc.sync.dma_start(out=xhiv[:, 1, isl], in_=xsrc_hi[:, 1, isl])
            nc.scalar.dma_start(out=w2b16[:, :],
                                in_=w2b_d.ap().rearrange(
                                    "p k t o -> p (k t o)"))
            nc.scalar.dma_start(out=par[:, :], in_=par_d.ap())
            for k in range(2):
                for hh in range(2):
                    isl = slice(hh * HB, (hh + 1) * HB)
                    nc.scalar.dma_start(out=xlov[:, k, isl],
                                        in_=xsrc_lo[:, k, isl])

            # derived weight planes (scalar; off critical path)
            nc.scalar.activation(out=w1bl[:, :], in_=w1b[:, :],
                                 func=AF.Copy, scale=2.0 ** -12)
            nc.scalar.activation(out=w2b8[:, :], in_=w2b16[:, :],
                                 func=AF.Copy)
            w2bv = w2b16.rearrange("p (k f) -> p k f", k=2)
            for k in range(2):
                zv = w2z[k].rearrange("p (k f) -> p k f", k=2)
                nc.gpsimd.memset(zv[:, 1 - k], 0.0)
                nc.scalar.activation(out=zv[:, k], in_=w2bv[:, k],
                                     func=AF.Copy)

            # zero ALL of y8 (not just the pad ring): the zero-weight
            # DoubleRow phases read the not-yet-signed half, and stray
            # bytes interpreted as fp8 NaN would poison PSUM (NaN*0=NaN)
            nc.gpsimd.memset(y8[:, :], 0.0)

            gcols = {1: (par[:, 0:2], par[:, 2:4], par[:, 4:6]),
                     2: (par[:, 6:8], par[:, 8:10], par[:, 10:12])}

            # ---- conv builders ----
            def conv_co(tag, planes, co, csb, sums, ssq):
                """planes: list of (weight_view[p,t*?o slice fn], moving
                view[p,i,h,w]) matmul'd in fp16."""
                NP = len(planes)
                pss = [PS.tile([128, CH_N], F32, tag=f"ps{ch}",
                               name=f"ps_{tag}_{co}_{ch}")
                       for ch in range(8)]

                def epilogue(ch):
                    cs = csb[co][:, ch * CH_N:(ch + 1) * CH_N]
                    sl = co * 8 + ch
                    nc.vector.tensor_scalar(
                        out=cs, in0=pss[ch][:, :], scalar1=0.0,
                        scalar2=0.0, op0=ALU.add, op1=ALU.add,
                        accum_out=sums[:, sl:sl + 1])
                    sq = SQ.tile([128, CH_N], F32, tag="sq",
                                 name=f"sq_{tag}_{co}_{ch}")
                    # square reads PSUM directly: runs parallel with the
                    # copy above instead of chained behind it
                    nc.scalar.activation(
                        out=sq[:, :], in_=pss[ch][:, :], func=AF.Square,
                        accum_out=ssq[:, sl:sl + 1])

                def emit(k, t, ch, im, h0):
                    dy, dx = t // 3, t % 3
                    wview, mview = planes[k]
                    wap = wview[:, t, co * 128:co * 128 + 128]
                    first = (k == 0 and t == 0)
                    last = (k == NP - 1 and t == KTAPS - 1)
                    mov = mview[:, im, h0 + dy:h0 + dy + CH_R, dx:dx + W]
                    nc.tensor.matmul(pss[ch][:, :], wap, mov,
                                     start=first, stop=last)

                for k in range(NP - 1):
                    for t in range(KTAPS):
                        for ch, (im, h0) in enumerate(CHUNKS):
                            emit(k, t, ch, im, h0)
                for ch, (im, h0) in enumerate(CHUNKS):
                    for t in range(KTAPS):
                        emit(NP - 1, t, ch, im, h0)
                    epilogue(ch)

            def c2_epilogue(co, pss, ch):
                cs = c2[co][:, ch * CH_N:(ch + 1) * CH_N]
                sl = co * 8 + ch
                nc.vector.tensor_scalar(
                    out=cs, in0=pss[ch][:, :], scalar1=0.0,
                    scalar2=0.0, op0=ALU.add, op1=ALU.add,
                    accum_out=sums2[:, sl:sl + 1])
                sq = SQ.tile([128, CH_N], F32, tag="sq",
                             name=f"sq_c2_{co}_{ch}")
                nc.scalar.activation(
                    out=sq[:, :], in_=pss[ch][:, :], func=AF.Square,
                    accum_out=ssq2[:, sl:sl + 1])

            def conv2_co_fp8(co):
                """DoubleRow fp8: both 128ch halves contracted per pass."""
                pss = [PS.tile([128, CH_N], F32, tag=f"ps{ch}",
                               name=f"ps_c2_{co}_{ch}")
                       for ch in range(8)]
                for ch, (im, h0) in enumerate(CHUNKS):
                    for t in range(KTAPS):
                        dy, dx = t // 3, t % 3
                        wap = w2v[:, :, t, co * 128:co * 128 + 128]
                        mov = y8v[:, :, im, h0 + dy:h0 + dy + CH_R,
                                  dx:dx + W]
                        nc.tensor.matmul(pss[ch][:, :], wap, mov,
                                         start=(t == 0), stop=(t == KTAPS - 1),
                                         perf_mode=DR_MODE)
                    c2_epilogue(co, pss, ch)

            # conv2-co0 as two full-DoubleRow phases with zero-padded weight
            # halves: phase 0 ([w_k0|0]) only truly depends on y[k0] (the k1
            # half reads memset zeros), so it runs during the AR2/y-co1
            # wait. MUST be emitted before y1_sign(1) so Tile doesn't chain
            # its y[k1]-range read behind the post-AR2 sign writes.
            w2zv = [w2z[k].rearrange("p (k t o) -> p k t o",
                                     k=2, t=KTAPS) for k in range(2)]
            pss_c0 = [PS.tile([128, CH_N], F32, tag=f"ps{ch}",
                              name=f"ps_c2_0_{ch}") for ch in range(8)]

            def conv2_co0_phase0():
                for t in range(KTAPS):
                    dy, dx = t // 3, t % 3
                    wap = w2zv[0][:, :, t, 0:128]
                    for ch, (im, h0) in enumerate(CHUNKS):
                        mov = y8v[:, :, im, h0 + dy:h0 + dy + CH_R, dx:dx + W]
                        nc.tensor.matmul(pss_c0[ch][:, :], wap, mov,
                                         start=(t == 0), stop=False,
                                         perf_mode=DR_MODE)

            def conv2_co0_phase1():
                for ch, (im, h0) in enumerate(CHUNKS):
                    for t in range(KTAPS):
                        dy, dx = t // 3, t % 3
                        wap = w2zv[1][:, :, t, 0:128]
                        mov = y8v[:, :, im, h0 + dy:h0 + dy + CH_R, dx:dx + W]
                        nc.tensor.matmul(pss_c0[ch][:, :], wap, mov,
                                         start=False, stop=(t == KTAPS - 1),
                                         perf_mode=DR_MODE)
                    c2_epilogue(0, pss_c0, ch)

            # ---- stats: reduce (vector) -> DMA (gpsimd) -> AR (gpsimd) ----
            def stats1_co(co):
                nc.vector.reduce_sum(out=st1[co][:, 0:1],
                                     in_=sums1[:, co * 8:(co + 1) * 8],
                                     axis=AX.X)
                nc.vector.reduce_sum(out=st1[co][:, 1:2],
                                     in_=ssq1[:, co * 8:(co + 1) * 8],
                                     axis=AX.X)
                # stats DMA on scalar = hardware DGE (~1us, vs gpsimd's
                # ~4us soft DGE) and NOT on sync: the sync queue holds the
                # AR readbacks, which complete at skew-inflated times — the
                # next AR's trigger must never chain behind them
                nc.scalar.dma_start(out=cc_in[co][:, :], in_=st1[co][:, :])
                nc.gpsimd.collective_compute(
                    "AllReduce", ALU.add,
                    replica_groups=[list(range(NCORES))],
                    ins=[cc_in[co].opt()], outs=[cc_out[co].opt()])
                # readback on sync (NOT gpsimd: keeps the next trigger free)
                nc.sync.dma_start(out=fin1[:, 2 * co:2 * co + 2],
                                  in_=cc_out[co][:, :])

            # ---- BN finalize: s,t columns (vector + one scalar sqrt) ----
            # co=None batches both co groups in 2-wide column ops
            def finalize_co(lay, co, fin, s_out, t_out, base, gate=None):
                gcol, bcol, ecol = gcols[lay]
                if co is None:
                    fv = fin.rearrange("p (c two) -> p c two", two=2)
                    Ssum = fv[:, :, 0]
                    Ssq = fv[:, :, 1]
                    nw = 2
                    so = s_out[:, :]
                    to = t_out[:, :]
                else:
                    Ssum = fin[:, 2 * co:2 * co + 1]
                    Ssq = fin[:, 2 * co + 1:2 * co + 2]
                    nw = 1
                    so = s_out[:, co:co + 1]
                    to = t_out[:, co:co + 1]
                    gcol = gcol[:, co:co + 1]
                    bcol = bcol[:, co:co + 1]
                    ecol = ecol[:, co:co + 1]
                mean = fsc[:, base + 0:base + nw]
                msq = fsc[:, base + 4:base + 4 + nw]
                u = fsc[:, base + 8:base + 8 + nw]
                tmp = fsc[:, base + 12:base + 12 + nw]
                if gate is not None:
                    # gate==0; its only purpose is a data dependency that
                    # keeps the Tile scheduler (whose sim has no idea how
                    # slow the AllReduce really is) from sequencing these
                    # chain heads ahead of still-pending conv epilogues.
                    # BOTH readback-reading head ops must be gated.
                    nc.vector.scalar_tensor_tensor(
                        out=mean, in0=Ssum, scalar=1.0 / NTOT,
                        in1=gate, op0=ALU.mult, op1=ALU.add)
                    nc.vector.scalar_tensor_tensor(
                        out=msq, in0=Ssq, scalar=1.0 / NTOT,
                        in1=gate, op0=ALU.mult, op1=ALU.add)
                else:
                    nc.vector.tensor_scalar(out=mean, in0=Ssum,
                                            scalar1=1.0 / NTOT,
                                            scalar2=None, op0=ALU.mult)
                    nc.vector.tensor_scalar(out=msq, in0=Ssq,
                                            scalar1=1.0 / NTOT,
                                            scalar2=None, op0=ALU.mult)
                nc.vector.tensor_tensor(out=u, in0=mean, in1=mean, op=ALU.mult)
                nc.vector.tensor_tensor(out=u, in0=msq, in1=u,
                                        op=ALU.subtract)
                nc.vector.tensor_tensor(out=u, in0=u, in1=ecol, op=ALU.add)
                # rsqrt = sqrt(1/u) + one Newton step
                nc.vector.reciprocal(out=so, in_=u)
                nc.scalar.activation(out=so, in_=so, func=AF.Sqrt)
                if NEWTON:
                    nc.vector.tensor_tensor(out=tmp, in0=so, in1=so,
                                            op=ALU.mult)
                    nc.vector.tensor_tensor(out=tmp, in0=tmp, in1=u,
                                            op=ALU.mult)
                    nc.vector.tensor_scalar(out=tmp, in0=tmp, scalar1=-0.5,
                                            scalar2=1.5, op0=ALU.mult,
                                            op1=ALU.add)
                    nc.vector.tensor_tensor(out=so, in0=so, in1=tmp,
                                            op=ALU.mult)
                nc.vector.tensor_tensor(out=so, in0=so, in1=gcol,
                                        op=ALU.mult)
                nc.vector.tensor_tensor(out=tmp, in0=so, in1=mean,
                                        op=ALU.mult)
                nc.vector.tensor_tensor(out=to, in0=bcol, in1=tmp,
                                        op=ALU.subtract)

            def y1_sign(co):
                # conv2's first chunk (img0 rows 0-13, all taps) needs y
                # rows 0-14 only: lead with a 16-row piece so the dependent
                # matmuls un-gate ~0.5us earlier, then img0's tail, then
                # the remaining images whole
                src = c1[co].rearrange("p (i h w) -> p i h w", i=BL, h=H, w=W)
                pieces = [(0, 0, 16), (0, 16, H)] + [(im, 0, H)
                                                    for im in range(1, BL)]
                for im, r0, r1 in pieces:
                    dst = y8v[:, co, im, 1 + r0:1 + r1, 1:WP - 1]
                    nc.scalar.activation(out=dst, in_=src[:, im, r0:r1],
                                         func=AF.Sign,
                                         bias=t1c[:, co:co + 1],
                                         scale=s1c[:, co:co + 1])

            planes1 = [(w1v[:, 0], xhiv[:, 0]), (w1v[:, 1], xhiv[:, 1]),
                       (w1lv[:, 0], xlov[:, 0]), (w1lv[:, 1], xlov[:, 1])]

            # ================= layer 1 =================
            def res_precompute(co):
                # res = x_hi + 2^-12 * x_lo  (== x to ~2^-22); per image so
                # the strided interior views stay 3D (walrus limit)
                rv = res[co].rearrange("p (i h w) -> p i h w", i=BL, h=H, w=W)
                for im in range(BL):
                    nc.vector.scalar_tensor_tensor(
                        out=rv[:, im],
                        in0=xlov[:, co, im, 1:HP - 1, 1:WP - 1],
                        scalar=2.0 ** -12,
                        in1=xhiv[:, co, im, 1:HP - 1, 1:WP - 1],
                        op0=ALU.mult, op1=ALU.add)

            conv_co("c1", planes1, 0, c1, sums1, ssq1)
            stats1_co(0)
            # residual precompute co0 (fills vector idle during AR1)
            res_precompute(0)
            conv_co("c1", planes1, 1, c1, sums1, ssq1)
            stats1_co(1)
            res_precompute(1)
            # scheduler gate: ready only once conv1-co1's stats are done
            nc.vector.tensor_scalar(out=gate1[:, :], in0=st1[1][:, 0:1],
                                    scalar1=0.0, scalar2=None, op0=ALU.mult)
            finalize_co(1, 0, fin1, s1c, t1c, 0, gate=gate1[:, 0:1])
            y1_sign(0)
            conv2_co0_phase0()
            finalize_co(1, 1, fin1, s1c, t1c, 1)
            y1_sign(1)

            # ================= layer 2 =================
            # one merged AllReduce for both co groups: consecutive CC ops
            # pay a ~20us stream re-arm, so a second L2 AR would start
            # re-arm-blocked and cost more than it hides
            def stats2_co(co):
                nc.vector.reduce_sum(out=st2[co][:, 0:1],
                                     in_=sums2[:, co * 8:(co + 1) * 8],
                                     axis=AX.X)
                nc.vector.reduce_sum(out=st2[co][:, 1:2],
                                     in_=ssq2[:, co * 8:(co + 1) * 8],
                                     axis=AX.X)

            outdst = out_d.ap().rearrange("i (k p) h w -> k p i (h w)", k=2)

            def out_epilogue(co, nsplit=2):
                # nsplit=4 for the last co group: tighter stt->Sign->DMA
                # pipeline on the exposed tail
                QSP = SP // nsplit
                QB = BL // nsplit if nsplit <= BL else 1
                for hh in range(nsplit):
                    sl = slice(hh * QSP, (hh + 1) * QSP)
                    nc.vector.scalar_tensor_tensor(
                        out=c2[co][:, sl], in0=c2[co][:, sl],
                        scalar=s2c[:, co:co + 1], in1=res[co][:, sl],
                        op0=ALU.mult, op1=ALU.add)
                    nc.scalar.activation(out=o16[co][:, sl],
                                         in_=c2[co][:, sl],
                                         func=AF.Sign, bias=t2c[:, co:co + 1])
                    nc.sync.dma_start(
                        out=outdst[co][:, hh * QB:(hh + 1) * QB],
                        in_=o16[co][:, sl].rearrange("p (i hw) -> p i hw",
                                                     i=QB))

            conv2_co0_phase1()
            stats2_co(0)
            # co0's stat half ships while conv2-co1 still runs
            nc.scalar.dma_start(out=cc_in2[:, 0:2], in_=st2[0][:, :])
            conv2_co_fp8(1)
            stats2_co(1)
            nc.scalar.dma_start(out=cc_in2[:, 2:4], in_=st2[1][:, :])
            nc.gpsimd.collective_compute(
                "AllReduce", ALU.add,
                replica_groups=[list(range(NCORES))],
                ins=[cc_in2.opt()], outs=[cc_out2.opt()])
            nc.sync.dma_start(out=fin2[:, :], in_=cc_out2[:, :])
            finalize_co(2, None, fin2, s2c, t2c, 2)
            out_epilogue(0)
            out_epilogue(1, nsplit=4)

    nc.compile()
    return nc


def _get_nc():
    if "nc" not in _NC_CACHE:
        _patch_ldw_opt()
        _NC_CACHE["nc"] = _build_nc()
    return _NC_CACHE["nc"]


def _host_prep(x, w1, tau1, gamma1, beta1, w2, tau2, gamma2, beta2):
    f = np.float32
    x = np.asarray(x, f)
    w1 = np.asarray(w1, f)
    w2 = np.asarray(w2, f)

    # hi/lo fp16 split, padded, partition-major [cores][128,2,BL,HP,WP]
    xhi = x.astype(np.float16)
    xlo = ((x - xhi.astype(f)) * 4096.0).astype(np.float16)

    def pack_x(a):
        pad = np.zeros((B, C, HP, WP), np.float16)
        pad[:, :, 1:HP - 1, 1:WP - 1] = a
        # [B,C,HP,WP] -> [cores, BL, k, p, HP, WP] -> [cores, p, k, BL, ...]
        r = pad.reshape(NCORES, BL, 2, 128, HP, WP).transpose(0, 3, 2, 1, 4, 5)
        return np.ascontiguousarray(r)

    xhi_p = pack_x(xhi)
    xlo_p = pack_x(xlo)

    def pack_w(w, tau):
        b = np.where(w > np.asarray(tau, f).reshape(C, 1, 1, 1), 1.0, -1.0)
        # [O,I,3,3] -> [I=k*128+p, t, O] -> [p, k, t, O]
        t = b.astype(np.float16).transpose(1, 2, 3, 0).reshape(2, 128,
                                                               KTAPS, C)
        return np.ascontiguousarray(t.transpose(1, 0, 2, 3))

    w1p = pack_w(w1, tau1)
    w2p = pack_w(w2, tau2)

    # fold alpha into eps:  s = gamma * rsqrt(var + eps/alpha^2)
    def epsp(w):
        a = np.abs(w.astype(np.float64)).mean(axis=(1, 2, 3))
        return (EPS / (a * a)).astype(f)

    cols = [np.asarray(gamma1, f).reshape(C), np.asarray(beta1, f).reshape(C),
            epsp(w1),
            np.asarray(gamma2, f).reshape(C), np.asarray(beta2, f).reshape(C),
            epsp(w2)]
    par = np.zeros((128, 12), f)
    for j, col in enumerate(cols):
        par[:, 2 * j] = col[0:128]
        par[:, 2 * j + 1] = col[128:256]

    return xhi_p, xlo_p, w1p, w2p, par


def kernel(x, w1, tau1, gamma1, beta1, w2, tau2, gamma2, beta2,
           trace=False, trace_kwargs=None):
    global LAST_RESULT
    xhi_p, xlo_p, w1p, w2p, par = _host_prep(
        x, w1, tau1, gamma1, beta1, w2, tau2, gamma2, beta2)
    common = {"w1b": w1p, "w2b": w2p, "par": par}
    in_maps = [
        {"xhi": xhi_p[i], "xlo": xlo_p[i], **common}
        for i in range(NCORES)
    ]
    nc = _get_nc()
    kwargs = {}
    if trace:
        kwargs["trace"] = True
        if trace_kwargs:
            kwargs.update(trace_kwargs)
    res = bass_utils.run_bass_kernel_spmd(nc, in_maps,
                                          core_ids=list(range(NCORES)),
                                          **kwargs)
    LAST_RESULT = res
    out = np.concatenate([res.results[i]["out"] for i in range(NCORES)],
                         axis=0)
    return out.astype(np.float32)
